# revision 35
# baseline (speedup 1.0000x reference)
"""MoE BasicRouter kernel for Trainium2 (Bass/Tile), 8-core SPMD.

Computes, for x [16384, 2048] f32, W [8, 2048] f32, b [8] f32:
  router_logits            [16384, 8]    f32   = x @ W.T + b
  expert_weights           [16384, 2]    f32   = top2(softmax(logits)) renormalized
  selected_expert_indices  [16384, 2]    int32
  expert_mask              [16384, 2, 8] int32 = one_hot(indices)

Sharding: data-parallel on the token dim across 8 NeuronCores; W and b are
replicated on every core.  Each core handles 2048 tokens.

Per-core structure (tokens mapped tile j, partition p -> token 16p+j so every
DRAM transfer is wide and per-partition contiguous):
  for each group g of 4 token tiles (512 tokens), contraction chunk c (128):
    - 4 PE transposes x[128t,128d] -> PSUM, ACT/DVE copies -> xtg [128d,512t]
    - 1 wide fp32 matmul  lgT[8,512] += WT_c.T @ xtg   (W stationary: the
      fp32 double weight-load is 8 columns, and N=512 keeps the PE warm)
  then per group: transpose lgT back to [t,e], +b, and run the top-2
  postprocessing + output DMAs while later groups still compute.
"""

import sys
from contextlib import ExitStack

import numpy as np

for _p in ("/opt/trn_rl_repo",):
    if _p not in sys.path:
        sys.path.insert(0, _p)

import concourse.bass as bass
import concourse.mybir as mybir
from concourse import bacc, bass_utils
from concourse.masks import make_identity
from concourse.tile import TileContext

N_CORES = 8
TOKENS = 16384
D = 2048
E = 8
TOPK = 2

T_CORE = TOKENS // N_CORES          # 2048 tokens per core
NT = T_CORE // 128                  # 16 token tiles of 128
NDC = D // 128                      # 16 contraction chunks of 128
GRP = 4                             # token tiles per logits matmul (N = 512)
NG = NT // GRP                      # 4 groups per core
NQ = 4                              # x sub-loads per token tile
QC = NDC // NQ                      # contraction chunks per quarter
DQ = D // NQ

FP32 = mybir.dt.float32
I32 = mybir.dt.int32
BIG = 1.0e6
NEG = -1.0e30
AF = mybir.ActivationFunctionType
OP = mybir.AluOpType

_CACHE = {}


def _build_program():
    """Trace the SPMD single-core program. Same program runs on all 8 cores."""
    nc = bacc.Bacc("TRN2", target_bir_lowering=False, debug=False)

    x_d = nc.dram_tensor("x", [T_CORE, D], FP32, kind="ExternalInput").ap()
    w_d = nc.dram_tensor("W", [E, D], FP32, kind="ExternalInput").ap()
    b_d = nc.dram_tensor("b", [E], FP32, kind="ExternalInput").ap()

    logits_d = nc.dram_tensor("router_logits", [T_CORE, E], FP32,
                              kind="ExternalOutput").ap()
    wout_d = nc.dram_tensor("expert_weights", [T_CORE, TOPK], FP32,
                            kind="ExternalOutput").ap()
    idx_d = nc.dram_tensor("selected_expert_indices", [T_CORE, TOPK], I32,
                           kind="ExternalOutput").ap()
    mask_d = nc.dram_tensor("expert_mask", [T_CORE, TOPK, E], I32,
                            kind="ExternalOutput").ap()

    # token (tile j, partition p) -> 16p + j, so per-partition runs are wide
    lg_view = logits_d.rearrange("(p s) e -> p s e", s=NT)
    w_view = wout_d.rearrange("(p s) k -> p s k", s=NT)
    i_view = idx_d.rearrange("(p s) k -> p s k", s=NT)
    m_view = mask_d.rearrange("(p s) k e -> p s k e", s=NT)
    x_view = x_d.rearrange("(p s) d -> s p d", s=NT)          # [16, 128, 2048]

    with TileContext(nc) as tc, ExitStack() as ctx:
        singles = ctx.enter_context(tc.tile_pool(name="singles", bufs=1))
        xpool = ctx.enter_context(tc.tile_pool(name="xpool", bufs=12))
        xtpool = ctx.enter_context(tc.tile_pool(name="xtpool", bufs=4))
        ppool = ctx.enter_context(tc.tile_pool(name="ppool", bufs=2))
        pspool = ctx.enter_context(tc.tile_pool(name="pspool", bufs=4,
                                                space="PSUM"))
        lgpool = ctx.enter_context(tc.tile_pool(name="lgpool", bufs=2,
                                                space="PSUM"))
        wtps = ctx.enter_context(tc.tile_pool(name="wtps", bufs=2,
                                              space="PSUM"))

        # ---- first x loads issued before anything else on the DMA queue -----
        x_quarts = {}

        def load_quarter(j, q):
            xq = xpool.tile([128, DQ], FP32, tag="x_tile", name=f"xq{j}_{q}")
            nc.sync.dma_start(out=xq, in_=x_view[j][:, q * DQ:(q + 1) * DQ])
            x_quarts[(j, q)] = xq

        for _jj in range(GRP):
            load_quarter(_jj, 0)

        # ---- one-time setup -------------------------------------------------
        ident = singles.tile([128, 128], FP32)
        make_identity(nc, ident)
        id8 = ident[0:E, 0:E]

        # b broadcast to all 128 partitions: [128, E]
        b_tile = singles.tile([128, E], FP32)
        b_bcast = bass.AP(tensor=b_d.tensor, offset=b_d.offset,
                          ap=[[0, 128]] + list(b_d.ap))
        nc.gpsimd.dma_start(out=b_tile, in_=b_bcast)

        w_sb = singles.tile([E, D], FP32)
        nc.sync.dma_start(out=w_sb, in_=w_d)

        # Dummy PE op that depends only on `ident`: advances PE's observed
        # gpsimd clock so the first real matmul below needs a single sync
        # wait (walrus rejects Matmults carrying 2 waits).
        ps_dummy = wtps.tile([8, 8], FP32, tag="ps_w")
        nc.tensor.transpose(ps_dummy, id8, id8)

        # HAM warmup: ~4us of real matmul activity while the first x DMAs are
        # still in flight, so the PE clock gate is at 2.4GHz when the real
        # work arrives.  Depends only on `ident`; runs during otherwise-idle
        # PE time.
        for wi in range(8):
            ps_warm = wtps.tile([128, 128], FP32, tag="ps_w",
                                name=f"ps_warm{wi}")
            nc.tensor.matmul(ps_warm, ident, ident)

        # W transposed into d-on-partition chunks: WT[:, c, :] = W[:, 128c+q].T
        wt_all = singles.tile([128, NDC, E], FP32)
        for c in range(NDC):
            ps_w = wtps.tile([128, E], FP32, tag="ps_w", name=f"ps_w{c}")
            nc.tensor.transpose(ps_w, w_sb[:, c * 128:(c + 1) * 128], id8)
            nc.vector.tensor_copy(wt_all[:, c, :], ps_w)

        # iota over experts, replicated on all partitions: [128, 1, E] f32
        iota8 = singles.tile([128, 1, E], FP32)
        for e in range(E):
            nc.vector.memset(iota8[:, :, e:e + 1], float(e))
        iota_b = iota8.to_broadcast([128, GRP, E])

        # ---- pipelined main loop --------------------------------------------
        LAG = 2
        n_steps = NG * NDC
        xtgs = [None] * n_steps
        lgTs = [None] * NG

        def emit_transposes(s):
            g, c = divmod(s, NDC)
            if c % QC == 0:
                # prefetch the quarter needed QC chunk-steps from now
                nxt = c + QC
                for jj in range(GRP):
                    if nxt < NDC:
                        load_quarter(g * GRP + jj, nxt // QC)
                    elif g + 1 < NG:
                        load_quarter((g + 1) * GRP + jj, 0)
            xtg = xtpool.tile([128, GRP, 128], FP32, tag="xtg", name=f"xtg{s}")
            for jj in range(GRP):
                if jj == 2 and s >= LAG:
                    # place the wide matmul mid-step: spreads real-MM
                    # activity so the HAM clock gate stays warm
                    emit_matmul(s - LAG)
                xq = x_quarts[(g * GRP + jj, c // QC)]
                cc = c % QC
                ps_t = pspool.tile([128, 128], FP32, tag="ps_t",
                                   name=f"ps_t{s}_{jj}")
                nc.tensor.transpose(ps_t, xq[:, cc * 128:(cc + 1) * 128],
                                    ident)
                if jj % 2 == 0:
                    nc.scalar.activation(xtg[:, jj, :], ps_t, AF.Copy)
                else:
                    nc.vector.tensor_copy(xtg[:, jj, :], ps_t)
            xtgs[s] = xtg

        def emit_post(g, Lg):
            """Top-2 + weights + indices + mask for one group of 512 tokens."""
            sl = slice(g * GRP, (g + 1) * GRP)

            m1 = ppool.tile([128, GRP, 1], FP32, tag="m1", name=f"m1_{g}")
            nc.vector.tensor_reduce(m1, Lg, axis=mybir.AxisListType.X,
                                    op=OP.max)
            cmp = ppool.tile([128, GRP, E], FP32, tag="cmp", name=f"cmp{g}")
            nc.vector.tensor_tensor(cmp, Lg, m1.to_broadcast([128, GRP, E]),
                                    op=OP.is_lt)
            tmp = ppool.tile([128, GRP, E], FP32, tag="tmp", name=f"tmp{g}")
            nc.vector.scalar_tensor_tensor(tmp, cmp, BIG, iota_b,
                                           op0=OP.mult, op1=OP.add)
            i1 = ppool.tile([128, GRP, 1], FP32, tag="i1", name=f"i1_{g}")
            nc.vector.tensor_reduce(i1, tmp, axis=mybir.AxisListType.X,
                                    op=OP.min)
            eq1 = ppool.tile([128, GRP, E], FP32, tag="eq1", name=f"eq1_{g}")
            nc.vector.tensor_tensor(eq1, iota_b,
                                    i1.to_broadcast([128, GRP, E]),
                                    op=OP.is_equal)
            L2 = ppool.tile([128, GRP, E], FP32, tag="L2", name=f"L2_{g}")
            nc.vector.scalar_tensor_tensor(L2, eq1, NEG, Lg,
                                           op0=OP.mult, op1=OP.add)
            m2 = ppool.tile([128, GRP, 1], FP32, tag="m2", name=f"m2_{g}")
            nc.vector.tensor_reduce(m2, L2, axis=mybir.AxisListType.X,
                                    op=OP.max)
            cmp2 = ppool.tile([128, GRP, E], FP32, tag="cmp", name=f"cmp2{g}")
            nc.vector.tensor_tensor(cmp2, L2, m2.to_broadcast([128, GRP, E]),
                                    op=OP.is_lt)
            tmp2 = ppool.tile([128, GRP, E], FP32, tag="tmp", name=f"tmp2{g}")
            nc.vector.scalar_tensor_tensor(tmp2, cmp2, BIG, iota_b,
                                           op0=OP.mult, op1=OP.add)
            i2 = ppool.tile([128, GRP, 1], FP32, tag="i2", name=f"i2_{g}")
            nc.vector.tensor_reduce(i2, tmp2, axis=mybir.AxisListType.X,
                                    op=OP.min)
            eq2 = ppool.tile([128, GRP, E], FP32, tag="eq2", name=f"eq2_{g}")
            nc.vector.tensor_tensor(eq2, iota_b,
                                    i2.to_broadcast([128, GRP, E]),
                                    op=OP.is_equal)

            # w1 = 1/(1+exp(m2-m1)), w2 = exp(m2-m1)/(1+exp(m2-m1)) -- equal to
            # the reference's renormalized top-2 softmax probabilities.
            delta = ppool.tile([128, GRP, 1], FP32, tag="delta",
                               name=f"delta{g}")
            nc.vector.tensor_tensor(delta, m2, m1, op=OP.subtract)
            e2 = ppool.tile([128, GRP, 1], FP32, tag="e2", name=f"e2_{g}")
            nc.scalar.activation(e2, delta, AF.Exp)
            den = ppool.tile([128, GRP, 1], FP32, tag="den", name=f"den{g}")
            nc.vector.tensor_scalar_add(den, e2, 1.0)
            w1 = ppool.tile([128, GRP, 1], FP32, tag="w1", name=f"w1_{g}")
            nc.vector.reciprocal(w1, den)
            w2 = ppool.tile([128, GRP, 1], FP32, tag="w2", name=f"w2_{g}")
            nc.vector.tensor_tensor(w2, e2, w1, op=OP.mult)

            w_out = ppool.tile([128, GRP, TOPK], FP32, tag="w_out",
                               name=f"w_out{g}")
            nc.vector.tensor_copy(w_out[:, :, 0:1], w1)
            nc.vector.tensor_copy(w_out[:, :, 1:2], w2)
            i_out = ppool.tile([128, GRP, TOPK], I32, tag="i_out",
                               name=f"i_out{g}")
            nc.vector.tensor_copy(i_out[:, :, 0:1], i1)
            nc.vector.tensor_copy(i_out[:, :, 1:2], i2)
            m_out = ppool.tile([128, GRP, TOPK, E], I32, tag="m_out",
                               name=f"m_out{g}")
            nc.vector.tensor_copy(m_out[:, :, 0, :], eq1)
            nc.vector.tensor_copy(m_out[:, :, 1, :], eq2)

            nc.sync.dma_start(out=lg_view[:, sl, :], in_=Lg)
            nc.sync.dma_start(out=w_view[:, sl, :], in_=w_out)
            nc.sync.dma_start(out=i_view[:, sl, :], in_=i_out)
            nc.sync.dma_start(out=m_view[:, sl, :, :], in_=m_out)

        def emit_matmul(s):
            g, c = divmod(s, NDC)
            if c == 0:
                lgTs[g] = lgpool.tile([E, GRP * 128], FP32, tag="lgT",
                                      name=f"lgT{g}")
            # lgT[e, t] += sum_d WT[d, e] * xT[d, t]
            nc.tensor.matmul(lgTs[g], wt_all[:, c, :],
                             xtgs[s].rearrange("p g t -> p (g t)"),
                             start=(c == 0), stop=(c == NDC - 1))
            if c == NDC - 1:
                # transpose lgT back to [tokens, experts] and add b
                lgT_sb = ppool.tile([E, GRP * 128], FP32, tag="lgT_sb",
                                    name=f"lgT_sb{g}")
                nc.vector.tensor_copy(lgT_sb, lgTs[g])
                Lg = ppool.tile([128, GRP, E], FP32, tag="Lg", name=f"Lg{g}")
                for jj in range(GRP):
                    ps_l = wtps.tile([128, E], FP32, tag="ps_w",
                                     name=f"ps_l{g}_{jj}")
                    nc.tensor.transpose(
                        ps_l, lgT_sb[:, jj * 128:(jj + 1) * 128], id8)
                    nc.vector.tensor_tensor(Lg[:, jj, :], ps_l, b_tile,
                                            op=OP.add)
                emit_post(g, Lg)

        for s in range(n_steps + LAG):
            if s < n_steps:
                emit_transposes(s)
            else:
                emit_matmul(s - LAG)

    nc.compile()
    return nc


def _get_program():
    if "nc" not in _CACHE:
        _CACHE["nc"] = _build_program()
    return _CACHE["nc"]


def run(x, W, b, trace=False):
    x = np.ascontiguousarray(np.asarray(x, dtype=np.float32))
    W = np.ascontiguousarray(np.asarray(W, dtype=np.float32))
    b = np.ascontiguousarray(np.asarray(b, dtype=np.float32))
    assert x.shape == (TOKENS, D) and W.shape == (E, D) and b.shape == (E,)

    nc = _get_program()
    in_maps = [
        {"x": x[c * T_CORE:(c + 1) * T_CORE], "W": W, "b": b}
        for c in range(N_CORES)
    ]
    res = bass_utils.run_bass_kernel_spmd(
        nc, in_maps, core_ids=list(range(N_CORES)), trace=trace,
    )
    _CACHE["last_result"] = res

    logits = np.concatenate(
        [res.results[c]["router_logits"] for c in range(N_CORES)], axis=0)
    weights = np.concatenate(
        [res.results[c]["expert_weights"] for c in range(N_CORES)], axis=0)
    idx = np.concatenate(
        [res.results[c]["selected_expert_indices"] for c in range(N_CORES)],
        axis=0).astype(np.int32)
    mask = np.concatenate(
        [res.results[c]["expert_mask"] for c in range(N_CORES)],
        axis=0).astype(np.int32)
    return logits, weights, idx, mask


def kernel(x, W, b):
    return run(x, W, b, trace=False)


# revision 36
# speedup vs baseline: 1.0158x; 1.0158x over previous
"""MoE BasicRouter kernel for Trainium2 (Bass/Tile), 8-core SPMD.

Computes, for x [16384, 2048] f32, W [8, 2048] f32, b [8] f32:
  router_logits            [16384, 8]    f32   = x @ W.T + b
  expert_weights           [16384, 2]    f32   = top2(softmax(logits)) renormalized
  selected_expert_indices  [16384, 2]    int32
  expert_mask              [16384, 2, 8] int32 = one_hot(indices)

Sharding: data-parallel on the token dim across 8 NeuronCores; W and b are
replicated on every core.  Each core handles 2048 tokens.

Per-core structure (tokens mapped tile j, partition p -> token 16p+j so every
DRAM transfer is wide and per-partition contiguous):
  for each group g of 4 token tiles (512 tokens), contraction chunk c (128):
    - 4 PE transposes x[128t,128d] -> PSUM, ACT/DVE copies -> xtg [128d,512t]
    - 1 wide fp32 matmul  lgT[8,512] += WT_c.T @ xtg   (W stationary: the
      fp32 double weight-load is 8 columns, and N=512 keeps the PE warm)
  then per group: transpose lgT back to [t,e], +b, and run the top-2
  postprocessing + output DMAs while later groups still compute.
"""

import sys
from contextlib import ExitStack

import numpy as np

for _p in ("/opt/trn_rl_repo",):
    if _p not in sys.path:
        sys.path.insert(0, _p)

import concourse.bass as bass
import concourse.mybir as mybir
from concourse import bacc, bass_utils
from concourse.masks import make_identity
from concourse.tile import TileContext

N_CORES = 8
TOKENS = 16384
D = 2048
E = 8
TOPK = 2

T_CORE = TOKENS // N_CORES          # 2048 tokens per core
NT = T_CORE // 128                  # 16 token tiles of 128
NDC = D // 128                      # 16 contraction chunks of 128
GRP = 4                             # token tiles per logits matmul (N = 512)
NG = NT // GRP                      # 4 groups per core
NQ = 4                              # x sub-loads per token tile
QC = NDC // NQ                      # contraction chunks per quarter
DQ = D // NQ

FP32 = mybir.dt.float32
I32 = mybir.dt.int32
BIG = 1.0e6
NEG = -1.0e30
AF = mybir.ActivationFunctionType
OP = mybir.AluOpType

_CACHE = {}


def _build_program():
    """Trace the SPMD single-core program. Same program runs on all 8 cores."""
    nc = bacc.Bacc("TRN2", target_bir_lowering=False, debug=False)

    x_d = nc.dram_tensor("x", [T_CORE, D], FP32, kind="ExternalInput").ap()
    w_d = nc.dram_tensor("W", [E, D], FP32, kind="ExternalInput").ap()
    b_d = nc.dram_tensor("b", [E], FP32, kind="ExternalInput").ap()

    logits_d = nc.dram_tensor("router_logits", [T_CORE, E], FP32,
                              kind="ExternalOutput").ap()
    wout_d = nc.dram_tensor("expert_weights", [T_CORE, TOPK], FP32,
                            kind="ExternalOutput").ap()
    idx_d = nc.dram_tensor("selected_expert_indices", [T_CORE, TOPK], I32,
                           kind="ExternalOutput").ap()
    mask_d = nc.dram_tensor("expert_mask", [T_CORE, TOPK, E], I32,
                            kind="ExternalOutput").ap()

    # token (tile j, partition p) -> 16p + j, so per-partition runs are wide
    lg_view = logits_d.rearrange("(p s) e -> p s e", s=NT)
    w_view = wout_d.rearrange("(p s) k -> p s k", s=NT)
    i_view = idx_d.rearrange("(p s) k -> p s k", s=NT)
    m_view = mask_d.rearrange("(p s) k e -> p s k e", s=NT)
    x_view = x_d.rearrange("(p s) d -> s p d", s=NT)          # [16, 128, 2048]

    with TileContext(nc) as tc, ExitStack() as ctx:
        singles = ctx.enter_context(tc.tile_pool(name="singles", bufs=1))
        xpool = ctx.enter_context(tc.tile_pool(name="xpool", bufs=12))
        xtpool = ctx.enter_context(tc.tile_pool(name="xtpool", bufs=4))
        ppool = ctx.enter_context(tc.tile_pool(name="ppool", bufs=2))
        pspool = ctx.enter_context(tc.tile_pool(name="pspool", bufs=4,
                                                space="PSUM"))
        lgpool = ctx.enter_context(tc.tile_pool(name="lgpool", bufs=2,
                                                space="PSUM"))
        wtps = ctx.enter_context(tc.tile_pool(name="wtps", bufs=2,
                                              space="PSUM"))

        # ---- first x loads issued before anything else on the DMA queue -----
        x_quarts = {}

        def load_quarter(j, q):
            xq = xpool.tile([128, DQ], FP32, tag="x_tile", name=f"xq{j}_{q}")
            nc.sync.dma_start(out=xq, in_=x_view[j][:, q * DQ:(q + 1) * DQ])
            x_quarts[(j, q)] = xq

        for _jj in range(GRP):
            load_quarter(_jj, 0)

        # ---- one-time setup -------------------------------------------------
        ident = singles.tile([128, 128], FP32)
        make_identity(nc, ident)
        id8 = ident[0:E, 0:E]

        # b broadcast to all 128 partitions: [128, E]
        b_tile = singles.tile([128, E], FP32)
        b_bcast = bass.AP(tensor=b_d.tensor, offset=b_d.offset,
                          ap=[[0, 128]] + list(b_d.ap))
        nc.gpsimd.dma_start(out=b_tile, in_=b_bcast)

        w_sb = singles.tile([E, D], FP32)
        nc.sync.dma_start(out=w_sb, in_=w_d)

        # Dummy PE op that depends only on `ident`: advances PE's observed
        # gpsimd clock so the first real matmul below needs a single sync
        # wait (walrus rejects Matmults carrying 2 waits).
        ps_dummy = wtps.tile([8, 8], FP32, tag="ps_w")
        nc.tensor.transpose(ps_dummy, id8, id8)

        # W transposed into d-on-partition chunks: WT[:, c, :] = W[:, 128c+q].T
        wt_all = singles.tile([128, NDC, E], FP32)
        for c in range(NDC):
            ps_w = wtps.tile([128, E], FP32, tag="ps_w", name=f"ps_w{c}")
            nc.tensor.transpose(ps_w, w_sb[:, c * 128:(c + 1) * 128], id8)
            nc.vector.tensor_copy(wt_all[:, c, :], ps_w)

        # iota over experts, replicated on all partitions: [128, 1, E] f32
        iota8 = singles.tile([128, 1, E], FP32)
        for e in range(E):
            nc.vector.memset(iota8[:, :, e:e + 1], float(e))
        iota_b = iota8.to_broadcast([128, GRP, E])

        # ---- pipelined main loop --------------------------------------------
        LAG = 2
        n_steps = NG * NDC
        xtgs = [None] * n_steps
        lgTs = [None] * NG

        def emit_transposes(s):
            g, c = divmod(s, NDC)
            if c % QC == 0:
                # prefetch the quarter needed QC chunk-steps from now
                nxt = c + QC
                for jj in range(GRP):
                    if nxt < NDC:
                        load_quarter(g * GRP + jj, nxt // QC)
                    elif g + 1 < NG:
                        load_quarter((g + 1) * GRP + jj, 0)
            xtg = xtpool.tile([128, GRP, 128], FP32, tag="xtg", name=f"xtg{s}")
            for jj in range(GRP):
                if jj == 2 and s >= LAG:
                    # place the wide matmul mid-step: spreads real-MM
                    # activity so the HAM clock gate stays warm
                    emit_matmul(s - LAG)
                xq = x_quarts[(g * GRP + jj, c // QC)]
                cc = c % QC
                ps_t = pspool.tile([128, 128], FP32, tag="ps_t",
                                   name=f"ps_t{s}_{jj}")
                nc.tensor.transpose(ps_t, xq[:, cc * 128:(cc + 1) * 128],
                                    ident)
                if jj % 2 == 0:
                    nc.scalar.activation(xtg[:, jj, :], ps_t, AF.Copy)
                else:
                    nc.vector.tensor_copy(xtg[:, jj, :], ps_t)
            xtgs[s] = xtg

        def emit_post(g, Lg):
            """Top-2 + weights + indices + mask for one group of 512 tokens."""
            sl = slice(g * GRP, (g + 1) * GRP)

            m1 = ppool.tile([128, GRP, 1], FP32, tag="m1", name=f"m1_{g}")
            nc.vector.tensor_reduce(m1, Lg, axis=mybir.AxisListType.X,
                                    op=OP.max)
            cmp = ppool.tile([128, GRP, E], FP32, tag="cmp", name=f"cmp{g}")
            nc.vector.tensor_tensor(cmp, Lg, m1.to_broadcast([128, GRP, E]),
                                    op=OP.is_lt)
            tmp = ppool.tile([128, GRP, E], FP32, tag="tmp", name=f"tmp{g}")
            nc.vector.scalar_tensor_tensor(tmp, cmp, BIG, iota_b,
                                           op0=OP.mult, op1=OP.add)
            i1 = ppool.tile([128, GRP, 1], FP32, tag="i1", name=f"i1_{g}")
            nc.vector.tensor_reduce(i1, tmp, axis=mybir.AxisListType.X,
                                    op=OP.min)
            eq1 = ppool.tile([128, GRP, E], FP32, tag="eq1", name=f"eq1_{g}")
            nc.vector.tensor_tensor(eq1, iota_b,
                                    i1.to_broadcast([128, GRP, E]),
                                    op=OP.is_equal)
            L2 = ppool.tile([128, GRP, E], FP32, tag="L2", name=f"L2_{g}")
            nc.vector.scalar_tensor_tensor(L2, eq1, NEG, Lg,
                                           op0=OP.mult, op1=OP.add)
            m2 = ppool.tile([128, GRP, 1], FP32, tag="m2", name=f"m2_{g}")
            nc.vector.tensor_reduce(m2, L2, axis=mybir.AxisListType.X,
                                    op=OP.max)
            cmp2 = ppool.tile([128, GRP, E], FP32, tag="cmp", name=f"cmp2{g}")
            nc.vector.tensor_tensor(cmp2, L2, m2.to_broadcast([128, GRP, E]),
                                    op=OP.is_lt)
            tmp2 = ppool.tile([128, GRP, E], FP32, tag="tmp", name=f"tmp2{g}")
            nc.vector.scalar_tensor_tensor(tmp2, cmp2, BIG, iota_b,
                                           op0=OP.mult, op1=OP.add)
            i2 = ppool.tile([128, GRP, 1], FP32, tag="i2", name=f"i2_{g}")
            nc.vector.tensor_reduce(i2, tmp2, axis=mybir.AxisListType.X,
                                    op=OP.min)
            eq2 = ppool.tile([128, GRP, E], FP32, tag="eq2", name=f"eq2_{g}")
            nc.vector.tensor_tensor(eq2, iota_b,
                                    i2.to_broadcast([128, GRP, E]),
                                    op=OP.is_equal)

            # w1 = 1/(1+exp(m2-m1)), w2 = exp(m2-m1)/(1+exp(m2-m1)) -- equal to
            # the reference's renormalized top-2 softmax probabilities.
            delta = ppool.tile([128, GRP, 1], FP32, tag="delta",
                               name=f"delta{g}")
            nc.vector.tensor_tensor(delta, m2, m1, op=OP.subtract)
            e2 = ppool.tile([128, GRP, 1], FP32, tag="e2", name=f"e2_{g}")
            nc.scalar.activation(e2, delta, AF.Exp)
            den = ppool.tile([128, GRP, 1], FP32, tag="den", name=f"den{g}")
            nc.vector.tensor_scalar_add(den, e2, 1.0)
            w1 = ppool.tile([128, GRP, 1], FP32, tag="w1", name=f"w1_{g}")
            nc.vector.reciprocal(w1, den)
            w2 = ppool.tile([128, GRP, 1], FP32, tag="w2", name=f"w2_{g}")
            nc.vector.tensor_tensor(w2, e2, w1, op=OP.mult)

            w_out = ppool.tile([128, GRP, TOPK], FP32, tag="w_out",
                               name=f"w_out{g}")
            nc.vector.tensor_copy(w_out[:, :, 0:1], w1)
            nc.vector.tensor_copy(w_out[:, :, 1:2], w2)
            i_out = ppool.tile([128, GRP, TOPK], I32, tag="i_out",
                               name=f"i_out{g}")
            nc.vector.tensor_copy(i_out[:, :, 0:1], i1)
            nc.vector.tensor_copy(i_out[:, :, 1:2], i2)
            m_out = ppool.tile([128, GRP, TOPK, E], I32, tag="m_out",
                               name=f"m_out{g}")
            nc.vector.tensor_copy(m_out[:, :, 0, :], eq1)
            nc.vector.tensor_copy(m_out[:, :, 1, :], eq2)

            nc.sync.dma_start(out=lg_view[:, sl, :], in_=Lg)
            nc.sync.dma_start(out=w_view[:, sl, :], in_=w_out)
            nc.sync.dma_start(out=i_view[:, sl, :], in_=i_out)
            nc.sync.dma_start(out=m_view[:, sl, :, :], in_=m_out)

        def emit_matmul(s):
            g, c = divmod(s, NDC)
            if c == 0:
                lgTs[g] = lgpool.tile([E, GRP * 128], FP32, tag="lgT",
                                      name=f"lgT{g}")
            # lgT[e, t] += sum_d WT[d, e] * xT[d, t]
            nc.tensor.matmul(lgTs[g], wt_all[:, c, :],
                             xtgs[s].rearrange("p g t -> p (g t)"),
                             start=(c == 0), stop=(c == NDC - 1))
            if c == NDC - 1:
                # transpose lgT back to [tokens, experts] and add b
                lgT_sb = ppool.tile([E, GRP * 128], FP32, tag="lgT_sb",
                                    name=f"lgT_sb{g}")
                nc.vector.tensor_copy(lgT_sb, lgTs[g])
                Lg = ppool.tile([128, GRP, E], FP32, tag="Lg", name=f"Lg{g}")
                for jj in range(GRP):
                    ps_l = wtps.tile([128, E], FP32, tag="ps_w",
                                     name=f"ps_l{g}_{jj}")
                    nc.tensor.transpose(
                        ps_l, lgT_sb[:, jj * 128:(jj + 1) * 128], id8)
                    nc.vector.tensor_tensor(Lg[:, jj, :], ps_l, b_tile,
                                            op=OP.add)
                emit_post(g, Lg)

        for s in range(n_steps + LAG):
            if s < n_steps:
                emit_transposes(s)
            else:
                emit_matmul(s - LAG)

    nc.compile()
    return nc


def _get_program():
    if "nc" not in _CACHE:
        _CACHE["nc"] = _build_program()
    return _CACHE["nc"]


def run(x, W, b, trace=False):
    x = np.ascontiguousarray(np.asarray(x, dtype=np.float32))
    W = np.ascontiguousarray(np.asarray(W, dtype=np.float32))
    b = np.ascontiguousarray(np.asarray(b, dtype=np.float32))
    assert x.shape == (TOKENS, D) and W.shape == (E, D) and b.shape == (E,)

    nc = _get_program()
    in_maps = [
        {"x": x[c * T_CORE:(c + 1) * T_CORE], "W": W, "b": b}
        for c in range(N_CORES)
    ]
    res = bass_utils.run_bass_kernel_spmd(
        nc, in_maps, core_ids=list(range(N_CORES)), trace=trace,
    )
    _CACHE["last_result"] = res

    logits = np.concatenate(
        [res.results[c]["router_logits"] for c in range(N_CORES)], axis=0)
    weights = np.concatenate(
        [res.results[c]["expert_weights"] for c in range(N_CORES)], axis=0)
    idx = np.concatenate(
        [res.results[c]["selected_expert_indices"] for c in range(N_CORES)],
        axis=0).astype(np.int32)
    mask = np.concatenate(
        [res.results[c]["expert_mask"] for c in range(N_CORES)],
        axis=0).astype(np.int32)
    return logits, weights, idx, mask


def kernel(x, W, b):
    return run(x, W, b, trace=False)


# revision 37
# speedup vs baseline: 1.0258x; 1.0098x over previous
"""MoE BasicRouter kernel for Trainium2 (Bass/Tile), 8-core SPMD.

Computes, for x [16384, 2048] f32, W [8, 2048] f32, b [8] f32:
  router_logits            [16384, 8]    f32   = x @ W.T + b
  expert_weights           [16384, 2]    f32   = top2(softmax(logits)) renormalized
  selected_expert_indices  [16384, 2]    int32
  expert_mask              [16384, 2, 8] int32 = one_hot(indices)

Sharding: data-parallel on the token dim across 8 NeuronCores; W and b are
replicated on every core.  Each core handles 2048 tokens.

Per-core structure (tokens mapped tile j, partition p -> token 16p+j so every
DRAM transfer is wide and per-partition contiguous):
  for each group g of 4 token tiles (512 tokens), contraction chunk c (128):
    - 4 PE transposes x[128t,128d] -> PSUM, ACT/DVE copies -> xtg [128d,512t]
    - 1 wide fp32 matmul  lgT[8,512] += WT_c.T @ xtg   (W stationary: the
      fp32 double weight-load is 8 columns, and N=512 keeps the PE warm)
  then per group: transpose lgT back to [t,e], +b, and run the top-2
  postprocessing + output DMAs while later groups still compute.
"""

import sys
from contextlib import ExitStack

import numpy as np

for _p in ("/opt/trn_rl_repo",):
    if _p not in sys.path:
        sys.path.insert(0, _p)

import concourse.bass as bass
import concourse.mybir as mybir
from concourse import bacc, bass_utils
from concourse.masks import make_identity
from concourse.tile import TileContext

N_CORES = 8
TOKENS = 16384
D = 2048
E = 8
TOPK = 2

T_CORE = TOKENS // N_CORES          # 2048 tokens per core
NT = T_CORE // 128                  # 16 token tiles of 128
NDC = D // 128                      # 16 contraction chunks of 128
GRP = 4                             # token tiles per logits matmul (N = 512)
NG = NT // GRP                      # 4 groups per core
NQ = 4                              # x sub-loads per token tile
QC = NDC // NQ                      # contraction chunks per quarter
DQ = D // NQ

FP32 = mybir.dt.float32
I32 = mybir.dt.int32
BIG = 1.0e6
NEG = -1.0e30
AF = mybir.ActivationFunctionType
OP = mybir.AluOpType

_CACHE = {}


def _build_program():
    """Trace the SPMD single-core program. Same program runs on all 8 cores."""
    nc = bacc.Bacc("TRN2", target_bir_lowering=False, debug=False)

    x_d = nc.dram_tensor("x", [T_CORE, D], FP32, kind="ExternalInput").ap()
    w_d = nc.dram_tensor("W", [E, D], FP32, kind="ExternalInput").ap()
    b_d = nc.dram_tensor("b", [E], FP32, kind="ExternalInput").ap()

    logits_d = nc.dram_tensor("router_logits", [T_CORE, E], FP32,
                              kind="ExternalOutput").ap()
    wout_d = nc.dram_tensor("expert_weights", [T_CORE, TOPK], FP32,
                            kind="ExternalOutput").ap()
    idx_d = nc.dram_tensor("selected_expert_indices", [T_CORE, TOPK], I32,
                           kind="ExternalOutput").ap()
    mask_d = nc.dram_tensor("expert_mask", [T_CORE, TOPK, E], I32,
                            kind="ExternalOutput").ap()

    # token (tile j, partition p) -> 16p + j, so per-partition runs are wide
    lg_view = logits_d.rearrange("(p s) e -> p s e", s=NT)
    w_view = wout_d.rearrange("(p s) k -> p s k", s=NT)
    i_view = idx_d.rearrange("(p s) k -> p s k", s=NT)
    m_view = mask_d.rearrange("(p s) k e -> p s k e", s=NT)
    x_view = x_d.rearrange("(p s) d -> s p d", s=NT)          # [16, 128, 2048]

    with TileContext(nc) as tc, ExitStack() as ctx:
        singles = ctx.enter_context(tc.tile_pool(name="singles", bufs=1))
        xpool = ctx.enter_context(tc.tile_pool(name="xpool", bufs=12))
        xtpool = ctx.enter_context(tc.tile_pool(name="xtpool", bufs=4))
        ppool = ctx.enter_context(tc.tile_pool(name="ppool", bufs=2))
        pspool = ctx.enter_context(tc.tile_pool(name="pspool", bufs=4,
                                                space="PSUM"))
        lgpool = ctx.enter_context(tc.tile_pool(name="lgpool", bufs=2,
                                                space="PSUM"))
        wtps = ctx.enter_context(tc.tile_pool(name="wtps", bufs=2,
                                              space="PSUM"))

        # ---- W first on the DMA queue (64KB; its PE setup then overlaps the
        # first x transfers), then the first x quarters ------------------------
        w_sb = singles.tile([E, D], FP32)
        nc.sync.dma_start(out=w_sb, in_=w_d)

        x_quarts = {}

        def load_quarter(j, q):
            xq = xpool.tile([128, DQ], FP32, tag="x_tile", name=f"xq{j}_{q}")
            nc.sync.dma_start(out=xq, in_=x_view[j][:, q * DQ:(q + 1) * DQ])
            x_quarts[(j, q)] = xq

        for _jj in range(GRP):
            load_quarter(_jj, 0)

        # ---- one-time setup -------------------------------------------------
        ident = singles.tile([128, 128], FP32)
        make_identity(nc, ident)
        id8 = ident[0:E, 0:E]

        # b broadcast to all 128 partitions: [128, E]
        b_tile = singles.tile([128, E], FP32)
        b_bcast = bass.AP(tensor=b_d.tensor, offset=b_d.offset,
                          ap=[[0, 128]] + list(b_d.ap))
        nc.gpsimd.dma_start(out=b_tile, in_=b_bcast)

        # Dummy PE op that depends only on `ident`: advances PE's observed
        # gpsimd clock so the first real matmul below needs a single sync
        # wait (walrus rejects Matmults carrying 2 waits).
        ps_dummy = wtps.tile([8, 8], FP32, tag="ps_w")
        nc.tensor.transpose(ps_dummy, id8, id8)

        # W transposed into d-on-partition chunks: WT[:, c, :] = W[:, 128c+q].T
        wt_all = singles.tile([128, NDC, E], FP32)
        for c in range(NDC):
            ps_w = wtps.tile([128, E], FP32, tag="ps_w", name=f"ps_w{c}")
            nc.tensor.transpose(ps_w, w_sb[:, c * 128:(c + 1) * 128], id8)
            nc.vector.tensor_copy(wt_all[:, c, :], ps_w)

        # iota over experts, replicated on all partitions: [128, 1, E] f32
        iota8 = singles.tile([128, 1, E], FP32)
        for e in range(E):
            nc.vector.memset(iota8[:, :, e:e + 1], float(e))
        iota_b = iota8.to_broadcast([128, GRP, E])

        # ---- pipelined main loop --------------------------------------------
        LAG = 2
        n_steps = NG * NDC
        xtgs = [None] * n_steps
        lgTs = [None] * NG

        def emit_transposes(s):
            g, c = divmod(s, NDC)
            if c % QC == 0:
                # prefetch the quarter needed QC chunk-steps from now
                nxt = c + QC
                for jj in range(GRP):
                    if nxt < NDC:
                        load_quarter(g * GRP + jj, nxt // QC)
                    elif g + 1 < NG:
                        load_quarter((g + 1) * GRP + jj, 0)
            xtg = xtpool.tile([128, GRP, 128], FP32, tag="xtg", name=f"xtg{s}")
            for jj in range(GRP):
                if jj == 2 and s >= LAG:
                    # place the wide matmul mid-step: spreads real-MM
                    # activity so the HAM clock gate stays warm
                    emit_matmul(s - LAG)
                xq = x_quarts[(g * GRP + jj, c // QC)]
                cc = c % QC
                ps_t = pspool.tile([128, 128], FP32, tag="ps_t",
                                   name=f"ps_t{s}_{jj}")
                nc.tensor.transpose(ps_t, xq[:, cc * 128:(cc + 1) * 128],
                                    ident)
                if jj % 2 == 0:
                    nc.scalar.activation(xtg[:, jj, :], ps_t, AF.Copy)
                else:
                    nc.vector.tensor_copy(xtg[:, jj, :], ps_t)
            xtgs[s] = xtg

        def emit_post(g, Lg):
            """Top-2 + weights + indices + mask for one group of 512 tokens."""
            sl = slice(g * GRP, (g + 1) * GRP)

            m1 = ppool.tile([128, GRP, 1], FP32, tag="m1", name=f"m1_{g}")
            nc.vector.tensor_reduce(m1, Lg, axis=mybir.AxisListType.X,
                                    op=OP.max)
            cmp = ppool.tile([128, GRP, E], FP32, tag="cmp", name=f"cmp{g}")
            nc.vector.tensor_tensor(cmp, Lg, m1.to_broadcast([128, GRP, E]),
                                    op=OP.is_lt)
            tmp = ppool.tile([128, GRP, E], FP32, tag="tmp", name=f"tmp{g}")
            nc.vector.scalar_tensor_tensor(tmp, cmp, BIG, iota_b,
                                           op0=OP.mult, op1=OP.add)
            i1 = ppool.tile([128, GRP, 1], FP32, tag="i1", name=f"i1_{g}")
            nc.vector.tensor_reduce(i1, tmp, axis=mybir.AxisListType.X,
                                    op=OP.min)
            eq1 = ppool.tile([128, GRP, E], FP32, tag="eq1", name=f"eq1_{g}")
            nc.vector.tensor_tensor(eq1, iota_b,
                                    i1.to_broadcast([128, GRP, E]),
                                    op=OP.is_equal)
            L2 = ppool.tile([128, GRP, E], FP32, tag="L2", name=f"L2_{g}")
            nc.vector.scalar_tensor_tensor(L2, eq1, NEG, Lg,
                                           op0=OP.mult, op1=OP.add)
            m2 = ppool.tile([128, GRP, 1], FP32, tag="m2", name=f"m2_{g}")
            nc.vector.tensor_reduce(m2, L2, axis=mybir.AxisListType.X,
                                    op=OP.max)
            cmp2 = ppool.tile([128, GRP, E], FP32, tag="cmp", name=f"cmp2{g}")
            nc.vector.tensor_tensor(cmp2, L2, m2.to_broadcast([128, GRP, E]),
                                    op=OP.is_lt)
            tmp2 = ppool.tile([128, GRP, E], FP32, tag="tmp", name=f"tmp2{g}")
            nc.vector.scalar_tensor_tensor(tmp2, cmp2, BIG, iota_b,
                                           op0=OP.mult, op1=OP.add)
            i2 = ppool.tile([128, GRP, 1], FP32, tag="i2", name=f"i2_{g}")
            nc.vector.tensor_reduce(i2, tmp2, axis=mybir.AxisListType.X,
                                    op=OP.min)
            eq2 = ppool.tile([128, GRP, E], FP32, tag="eq2", name=f"eq2_{g}")
            nc.vector.tensor_tensor(eq2, iota_b,
                                    i2.to_broadcast([128, GRP, E]),
                                    op=OP.is_equal)

            # w1 = 1/(1+exp(m2-m1)), w2 = exp(m2-m1)/(1+exp(m2-m1)) -- equal to
            # the reference's renormalized top-2 softmax probabilities.
            delta = ppool.tile([128, GRP, 1], FP32, tag="delta",
                               name=f"delta{g}")
            nc.vector.tensor_tensor(delta, m2, m1, op=OP.subtract)
            e2 = ppool.tile([128, GRP, 1], FP32, tag="e2", name=f"e2_{g}")
            nc.scalar.activation(e2, delta, AF.Exp)
            den = ppool.tile([128, GRP, 1], FP32, tag="den", name=f"den{g}")
            nc.vector.tensor_scalar_add(den, e2, 1.0)
            w1 = ppool.tile([128, GRP, 1], FP32, tag="w1", name=f"w1_{g}")
            nc.vector.reciprocal(w1, den)
            w2 = ppool.tile([128, GRP, 1], FP32, tag="w2", name=f"w2_{g}")
            nc.vector.tensor_tensor(w2, e2, w1, op=OP.mult)

            w_out = ppool.tile([128, GRP, TOPK], FP32, tag="w_out",
                               name=f"w_out{g}")
            nc.vector.tensor_copy(w_out[:, :, 0:1], w1)
            nc.vector.tensor_copy(w_out[:, :, 1:2], w2)
            i_out = ppool.tile([128, GRP, TOPK], I32, tag="i_out",
                               name=f"i_out{g}")
            nc.vector.tensor_copy(i_out[:, :, 0:1], i1)
            nc.vector.tensor_copy(i_out[:, :, 1:2], i2)
            m_out = ppool.tile([128, GRP, TOPK, E], I32, tag="m_out",
                               name=f"m_out{g}")
            nc.vector.tensor_copy(m_out[:, :, 0, :], eq1)
            nc.vector.tensor_copy(m_out[:, :, 1, :], eq2)

            nc.sync.dma_start(out=lg_view[:, sl, :], in_=Lg)
            nc.sync.dma_start(out=w_view[:, sl, :], in_=w_out)
            nc.sync.dma_start(out=i_view[:, sl, :], in_=i_out)
            nc.sync.dma_start(out=m_view[:, sl, :, :], in_=m_out)

        def emit_matmul(s):
            g, c = divmod(s, NDC)
            if c == 0:
                lgTs[g] = lgpool.tile([E, GRP * 128], FP32, tag="lgT",
                                      name=f"lgT{g}")
            # lgT[e, t] += sum_d WT[d, e] * xT[d, t]
            nc.tensor.matmul(lgTs[g], wt_all[:, c, :],
                             xtgs[s].rearrange("p g t -> p (g t)"),
                             start=(c == 0), stop=(c == NDC - 1))
            if c == NDC - 1:
                # transpose lgT back to [tokens, experts] and add b
                lgT_sb = ppool.tile([E, GRP * 128], FP32, tag="lgT_sb",
                                    name=f"lgT_sb{g}")
                nc.vector.tensor_copy(lgT_sb, lgTs[g])
                Lg = ppool.tile([128, GRP, E], FP32, tag="Lg", name=f"Lg{g}")
                for jj in range(GRP):
                    ps_l = wtps.tile([128, E], FP32, tag="ps_w",
                                     name=f"ps_l{g}_{jj}")
                    nc.tensor.transpose(
                        ps_l, lgT_sb[:, jj * 128:(jj + 1) * 128], id8)
                    nc.vector.tensor_tensor(Lg[:, jj, :], ps_l, b_tile,
                                            op=OP.add)
                emit_post(g, Lg)

        for s in range(n_steps + LAG):
            if s < n_steps:
                emit_transposes(s)
            else:
                emit_matmul(s - LAG)

    nc.compile()
    return nc


def _get_program():
    if "nc" not in _CACHE:
        _CACHE["nc"] = _build_program()
    return _CACHE["nc"]


def run(x, W, b, trace=False):
    x = np.ascontiguousarray(np.asarray(x, dtype=np.float32))
    W = np.ascontiguousarray(np.asarray(W, dtype=np.float32))
    b = np.ascontiguousarray(np.asarray(b, dtype=np.float32))
    assert x.shape == (TOKENS, D) and W.shape == (E, D) and b.shape == (E,)

    nc = _get_program()
    in_maps = [
        {"x": x[c * T_CORE:(c + 1) * T_CORE], "W": W, "b": b}
        for c in range(N_CORES)
    ]
    res = bass_utils.run_bass_kernel_spmd(
        nc, in_maps, core_ids=list(range(N_CORES)), trace=trace,
    )
    _CACHE["last_result"] = res

    logits = np.concatenate(
        [res.results[c]["router_logits"] for c in range(N_CORES)], axis=0)
    weights = np.concatenate(
        [res.results[c]["expert_weights"] for c in range(N_CORES)], axis=0)
    idx = np.concatenate(
        [res.results[c]["selected_expert_indices"] for c in range(N_CORES)],
        axis=0).astype(np.int32)
    mask = np.concatenate(
        [res.results[c]["expert_mask"] for c in range(N_CORES)],
        axis=0).astype(np.int32)
    return logits, weights, idx, mask


def kernel(x, W, b):
    return run(x, W, b, trace=False)


# revision 40
# speedup vs baseline: 1.0320x; 1.0061x over previous
"""MoE BasicRouter kernel for Trainium2 (Bass/Tile), 8-core SPMD.

Computes, for x [16384, 2048] f32, W [8, 2048] f32, b [8] f32:
  router_logits            [16384, 8]    f32   = x @ W.T + b
  expert_weights           [16384, 2]    f32   = top2(softmax(logits)) renormalized
  selected_expert_indices  [16384, 2]    int32
  expert_mask              [16384, 2, 8] int32 = one_hot(indices)

Sharding: data-parallel on the token dim across 8 NeuronCores; W and b are
replicated on every core.  Each core handles 2048 tokens.

Per-core structure (tokens mapped tile j, partition p -> token 16p+j so every
DRAM transfer is wide and per-partition contiguous):
  for each group g of 4 token tiles (512 tokens), contraction chunk c (128):
    - 4 PE transposes x[128t,128d] -> PSUM, ACT/DVE copies -> xtg [128d,512t]
    - 1 wide fp32 matmul  lgT[8,512] += WT_c.T @ xtg   (W stationary: the
      fp32 double weight-load is 8 columns, and N=512 keeps the PE warm)
  then per group: transpose lgT back to [t,e], +b, and run the top-2
  postprocessing + output DMAs while later groups still compute.
"""

import sys
from contextlib import ExitStack

import numpy as np

for _p in ("/opt/trn_rl_repo",):
    if _p not in sys.path:
        sys.path.insert(0, _p)

import concourse.bass as bass
import concourse.mybir as mybir
from concourse import bacc, bass_utils
from concourse.masks import make_identity
from concourse.tile import TileContext

N_CORES = 8
TOKENS = 16384
D = 2048
E = 8
TOPK = 2

T_CORE = TOKENS // N_CORES          # 2048 tokens per core
NT = T_CORE // 128                  # 16 token tiles of 128
NDC = D // 128                      # 16 contraction chunks of 128
GRP = 4                             # token tiles per logits matmul (N = 512)
NG = NT // GRP                      # 4 groups per core
NQ = 4                              # x sub-loads per token tile
QC = NDC // NQ                      # contraction chunks per quarter
DQ = D // NQ

FP32 = mybir.dt.float32
I32 = mybir.dt.int32
BIG = 1.0e6
NEG = -1.0e30
AF = mybir.ActivationFunctionType
OP = mybir.AluOpType

_CACHE = {}


def _build_program():
    """Trace the SPMD single-core program. Same program runs on all 8 cores."""
    nc = bacc.Bacc("TRN2", target_bir_lowering=False, debug=False)

    x_d = nc.dram_tensor("x", [T_CORE, D], FP32, kind="ExternalInput").ap()
    w_d = nc.dram_tensor("W", [E, D], FP32, kind="ExternalInput").ap()
    b_d = nc.dram_tensor("b", [E], FP32, kind="ExternalInput").ap()

    logits_d = nc.dram_tensor("router_logits", [T_CORE, E], FP32,
                              kind="ExternalOutput").ap()
    wout_d = nc.dram_tensor("expert_weights", [T_CORE, TOPK], FP32,
                            kind="ExternalOutput").ap()
    idx_d = nc.dram_tensor("selected_expert_indices", [T_CORE, TOPK], I32,
                           kind="ExternalOutput").ap()
    mask_d = nc.dram_tensor("expert_mask", [T_CORE, TOPK, E], I32,
                            kind="ExternalOutput").ap()

    # token (tile j, partition p) -> 16p + j, so per-partition runs are wide
    lg_view = logits_d.rearrange("(p s) e -> p s e", s=NT)
    w_view = wout_d.rearrange("(p s) k -> p s k", s=NT)
    i_view = idx_d.rearrange("(p s) k -> p s k", s=NT)
    m_view = mask_d.rearrange("(p s) k e -> p s k e", s=NT)
    x_view = x_d.rearrange("(p s) d -> s p d", s=NT)          # [16, 128, 2048]

    with TileContext(nc) as tc, ExitStack() as ctx:
        singles = ctx.enter_context(tc.tile_pool(name="singles", bufs=1))
        xpool = ctx.enter_context(tc.tile_pool(name="xpool", bufs=12))
        xtpool = ctx.enter_context(tc.tile_pool(name="xtpool", bufs=4))
        ppool = ctx.enter_context(tc.tile_pool(name="ppool", bufs=2))
        pspool = ctx.enter_context(tc.tile_pool(name="pspool", bufs=4,
                                                space="PSUM"))
        lgpool = ctx.enter_context(tc.tile_pool(name="lgpool", bufs=2,
                                                space="PSUM"))
        wtps = ctx.enter_context(tc.tile_pool(name="wtps", bufs=2,
                                              space="PSUM"))

        # ---- W first on the DMA queue (64KB; its PE setup then overlaps the
        # first x transfers), then the first x quarters ------------------------
        w_sb = singles.tile([E, D], FP32)
        nc.sync.dma_start(out=w_sb, in_=w_d)

        x_quarts = {}

        def load_quarter(j, q):
            xq = xpool.tile([128, DQ], FP32, tag="x_tile", name=f"xq{j}_{q}")
            nc.sync.dma_start(out=xq, in_=x_view[j][:, q * DQ:(q + 1) * DQ])
            x_quarts[(j, q)] = xq

        for _jj in range(GRP):
            load_quarter(_jj, 0)

        # ---- one-time setup -------------------------------------------------
        ident = singles.tile([128, 128], FP32)
        make_identity(nc, ident)
        id8 = ident[0:E, 0:E]

        # b broadcast to all 128 partitions: [128, E]
        b_tile = singles.tile([128, E], FP32)
        b_bcast = bass.AP(tensor=b_d.tensor, offset=b_d.offset,
                          ap=[[0, 128]] + list(b_d.ap))
        nc.gpsimd.dma_start(out=b_tile, in_=b_bcast)

        # Dummy PE op that depends only on `ident`: advances PE's observed
        # gpsimd clock so the first real matmul below needs a single sync
        # wait (walrus rejects Matmults carrying 2 waits).
        ps_dummy = wtps.tile([8, 8], FP32, tag="ps_w")
        nc.tensor.transpose(ps_dummy, id8, id8)

        # W transposed into d-on-partition chunks: WT[:, c, :] = W[:, 128c+q].T
        wt_all = singles.tile([128, NDC, E], FP32)
        for c in range(NDC):
            ps_w = wtps.tile([128, E], FP32, tag="ps_w", name=f"ps_w{c}")
            nc.tensor.transpose(ps_w, w_sb[:, c * 128:(c + 1) * 128], id8)
            nc.vector.tensor_copy(wt_all[:, c, :], ps_w)

        # iota over experts, replicated on all partitions: [128, 1, E] f32
        iota8 = singles.tile([128, 1, E], FP32)
        for e in range(E):
            nc.vector.memset(iota8[:, :, e:e + 1], float(e))
        iota_b = iota8.to_broadcast([128, GRP, E])

        # ---- pipelined main loop --------------------------------------------
        LAG = 2
        n_steps = NG * NDC
        xtgs = [None] * n_steps
        lgTs = [None] * NG

        def emit_transposes(s):
            g, c = divmod(s, NDC)
            if c % QC == 0:
                # prefetch the quarter needed QC chunk-steps from now
                nxt = c + QC
                for jj in range(GRP):
                    if nxt < NDC:
                        load_quarter(g * GRP + jj, nxt // QC)
                    elif g + 1 < NG:
                        load_quarter((g + 1) * GRP + jj, 0)
            xtg = xtpool.tile([128, GRP, 128], FP32, tag="xtg", name=f"xtg{s}")
            for jj in range(GRP):
                if jj == 2 and s >= LAG:
                    # place the wide matmul mid-step: spreads real-MM
                    # activity so the HAM clock gate stays warm
                    emit_matmul(s - LAG)
                xq = x_quarts[(g * GRP + jj, c // QC)]
                cc = c % QC
                ps_t = pspool.tile([128, 128], FP32, tag="ps_t",
                                   name=f"ps_t{s}_{jj}")
                nc.tensor.transpose(ps_t, xq[:, cc * 128:(cc + 1) * 128],
                                    ident)
                if jj % 2 == 0:
                    nc.scalar.activation(xtg[:, jj, :], ps_t, AF.Copy)
                else:
                    nc.vector.tensor_copy(xtg[:, jj, :], ps_t)
            xtgs[s] = xtg

        def emit_post(g, Lg):
            """Top-2 + weights + indices + mask for one group of 512 tokens."""
            sl = slice(g * GRP, (g + 1) * GRP)

            m1 = ppool.tile([128, GRP, 1], FP32, tag="m1", name=f"m1_{g}")
            nc.vector.tensor_reduce(m1, Lg, axis=mybir.AxisListType.X,
                                    op=OP.max)
            cmp = ppool.tile([128, GRP, E], FP32, tag="cmp", name=f"cmp{g}")
            nc.vector.tensor_tensor(cmp, Lg, m1.to_broadcast([128, GRP, E]),
                                    op=OP.is_lt)
            tmp = ppool.tile([128, GRP, E], FP32, tag="tmp", name=f"tmp{g}")
            nc.vector.scalar_tensor_tensor(tmp, cmp, BIG, iota_b,
                                           op0=OP.mult, op1=OP.add)
            i1 = ppool.tile([128, GRP, 1], FP32, tag="i1", name=f"i1_{g}")
            nc.vector.tensor_reduce(i1, tmp, axis=mybir.AxisListType.X,
                                    op=OP.min)
            eq1 = ppool.tile([128, GRP, E], FP32, tag="eq1", name=f"eq1_{g}")
            nc.vector.tensor_tensor(eq1, iota_b,
                                    i1.to_broadcast([128, GRP, E]),
                                    op=OP.is_equal)
            L2 = ppool.tile([128, GRP, E], FP32, tag="L2", name=f"L2_{g}")
            nc.vector.scalar_tensor_tensor(L2, eq1, NEG, Lg,
                                           op0=OP.mult, op1=OP.add)
            m2 = ppool.tile([128, GRP, 1], FP32, tag="m2", name=f"m2_{g}")
            nc.vector.tensor_reduce(m2, L2, axis=mybir.AxisListType.X,
                                    op=OP.max)
            cmp2 = ppool.tile([128, GRP, E], FP32, tag="cmp", name=f"cmp2{g}")
            nc.vector.tensor_tensor(cmp2, L2, m2.to_broadcast([128, GRP, E]),
                                    op=OP.is_lt)
            tmp2 = ppool.tile([128, GRP, E], FP32, tag="tmp", name=f"tmp2{g}")
            nc.vector.scalar_tensor_tensor(tmp2, cmp2, BIG, iota_b,
                                           op0=OP.mult, op1=OP.add)
            i2 = ppool.tile([128, GRP, 1], FP32, tag="i2", name=f"i2_{g}")
            nc.vector.tensor_reduce(i2, tmp2, axis=mybir.AxisListType.X,
                                    op=OP.min)
            eq2 = ppool.tile([128, GRP, E], FP32, tag="eq2", name=f"eq2_{g}")
            nc.vector.tensor_tensor(eq2, iota_b,
                                    i2.to_broadcast([128, GRP, E]),
                                    op=OP.is_equal)

            # w1 = 1/(1+exp(m2-m1)), w2 = exp(m2-m1)/(1+exp(m2-m1)) -- equal to
            # the reference's renormalized top-2 softmax probabilities.
            delta = ppool.tile([128, GRP, 1], FP32, tag="delta",
                               name=f"delta{g}")
            nc.vector.tensor_tensor(delta, m2, m1, op=OP.subtract)
            e2 = ppool.tile([128, GRP, 1], FP32, tag="e2", name=f"e2_{g}")
            nc.scalar.activation(e2, delta, AF.Exp)
            den = ppool.tile([128, GRP, 1], FP32, tag="den", name=f"den{g}")
            nc.vector.tensor_scalar_add(den, e2, 1.0)
            w1 = ppool.tile([128, GRP, 1], FP32, tag="w1", name=f"w1_{g}")
            nc.vector.reciprocal(w1, den)
            w2 = ppool.tile([128, GRP, 1], FP32, tag="w2", name=f"w2_{g}")
            nc.vector.tensor_tensor(w2, e2, w1, op=OP.mult)

            w_out = ppool.tile([128, GRP, TOPK], FP32, tag="w_out",
                               name=f"w_out{g}")
            nc.vector.tensor_copy(w_out[:, :, 0:1], w1)
            nc.vector.tensor_copy(w_out[:, :, 1:2], w2)
            i_out = ppool.tile([128, GRP, TOPK], I32, tag="i_out",
                               name=f"i_out{g}")
            nc.vector.tensor_copy(i_out[:, :, 0:1], i1)
            nc.vector.tensor_copy(i_out[:, :, 1:2], i2)
            m_out = ppool.tile([128, GRP, TOPK, E], I32, tag="m_out",
                               name=f"m_out{g}")
            nc.vector.tensor_copy(m_out[:, :, 0, :], eq1)
            nc.vector.tensor_copy(m_out[:, :, 1, :], eq2)

            nc.sync.dma_start(out=lg_view[:, sl, :], in_=Lg)
            nc.sync.dma_start(out=w_view[:, sl, :], in_=w_out)
            nc.sync.dma_start(out=i_view[:, sl, :], in_=i_out)
            nc.sync.dma_start(out=m_view[:, sl, :, :], in_=m_out)

        def emit_matmul(s):
            g, c = divmod(s, NDC)
            if c == 0:
                lgTs[g] = lgpool.tile([E, GRP * 128], FP32, tag="lgT",
                                      name=f"lgT{g}")
            # lgT[e, t] += sum_d WT[d, e] * xT[d, t]
            nc.tensor.matmul(lgTs[g], wt_all[:, c, :],
                             xtgs[s].rearrange("p g t -> p (g t)"),
                             start=(c == 0), stop=(c == NDC - 1))
            if c == NDC - 1:
                # transpose lgT back to [tokens, experts] and add b
                lgT_sb = ppool.tile([E, GRP * 128], FP32, tag="lgT_sb",
                                    name=f"lgT_sb{g}")
                nc.vector.tensor_copy(lgT_sb, lgTs[g])
                Lg = ppool.tile([128, GRP, E], FP32, tag="Lg", name=f"Lg{g}")
                for jj in range(GRP):
                    ps_l = wtps.tile([128, E], FP32, tag="ps_w",
                                     name=f"ps_l{g}_{jj}")
                    nc.tensor.transpose(
                        ps_l, lgT_sb[:, jj * 128:(jj + 1) * 128], id8)
                    nc.vector.tensor_tensor(Lg[:, jj, :], ps_l, b_tile,
                                            op=OP.add)
                emit_post(g, Lg)

        for s in range(n_steps + LAG):
            if s < n_steps:
                emit_transposes(s)
            else:
                emit_matmul(s - LAG)

    nc.compile()
    return nc


def _get_program():
    if "nc" not in _CACHE:
        _CACHE["nc"] = _build_program()
    return _CACHE["nc"]


def run(x, W, b, trace=False):
    x = np.ascontiguousarray(np.asarray(x, dtype=np.float32))
    W = np.ascontiguousarray(np.asarray(W, dtype=np.float32))
    b = np.ascontiguousarray(np.asarray(b, dtype=np.float32))
    assert x.shape == (TOKENS, D) and W.shape == (E, D) and b.shape == (E,)

    nc = _get_program()
    in_maps = [
        {"x": x[c * T_CORE:(c + 1) * T_CORE], "W": W, "b": b}
        for c in range(N_CORES)
    ]
    res = bass_utils.run_bass_kernel_spmd(
        nc, in_maps, core_ids=list(range(N_CORES)), trace=trace,
    )
    _CACHE["last_result"] = res

    logits = np.concatenate(
        [res.results[c]["router_logits"] for c in range(N_CORES)], axis=0)
    weights = np.concatenate(
        [res.results[c]["expert_weights"] for c in range(N_CORES)], axis=0)
    idx = np.concatenate(
        [res.results[c]["selected_expert_indices"] for c in range(N_CORES)],
        axis=0).astype(np.int32)
    mask = np.concatenate(
        [res.results[c]["expert_mask"] for c in range(N_CORES)],
        axis=0).astype(np.int32)
    return logits, weights, idx, mask


def kernel(x, W, b):
    return run(x, W, b, trace=False)
